# revision 17
# baseline (speedup 1.0000x reference)
"""LSTMCell (B=16384, IN=HID=512) on 8 TRN2 NeuronCores.

Strategy: data-parallel over batch (2048 rows/core), weights replicated.
Host pre-packs operands so the device kernel needs zero transposes:
  - GEMM computed as gates.T = W_cat.T @ [x;h].T  (K=1024 on partitions)
  - x/h/W cast to bf16 on host (fp32 PSUM accumulation on PE)
  - c / outputs stay fp32

Schedule (v4): the PE floor for this GEMM is 512 MMs x 512 cols ~= 109us
at bf16; the wins over the v1 baseline are at the edges:
  - batch-chunk 0 (r0+r1) runs K-OUTER across all 8 PSUM banks,
    consuming 512KB weight k-chunks as they stream in, so real matmuls
    start once w[k=0] + xh0[k<2] land instead of waiting for the full
    4MB weight load.
  - DMA bandwidth share is proportional to per-partition line size and
    queues drain in order, so: every bulk tensor keeps 4-8KB lines, the
    weight chunks alternate across the sync and gpsimd queues (double
    share during the critical window), and late-use inputs are queued
    BEHIND the weights on the same queues instead of contending.
  - warmup MMs depend only on a tiny memset, covering the HAM cold-clock
    window during the first DMAs without delaying real work.
  - per-group gate order (f,i,g,o) with eagerly emitted ACT/DVE chain:
    eviction order matches the next group's bank-demand order, and the
    end-of-kernel tail shrinks from ~6.7us to ~2us.
"""

import sys

sys.path.insert(0, "/opt/trn_rl_repo")

from contextlib import ExitStack

import ml_dtypes
import numpy as np

import concourse.bass as bass  # noqa: F401  (bass types used via bacc/mybir)
import concourse.mybir as mybir
import concourse.tile as tile
from concourse import bacc
from concourse.bass_utils import run_bass_kernel_spmd

B_FULL, IN, HID = 16384, 512, 512
NCORES = 8
BL = B_FULL // NCORES  # 2048 batch rows per core
JW = 512               # batch columns per chunk (matmul free dim)
P = 128

BF16 = mybir.dt.bfloat16
F32 = mybir.dt.float32
AF = mybir.ActivationFunctionType
BF16_NP = ml_dtypes.bfloat16

NK = (IN + HID) // P   # 8  k-chunks of the contraction dim
NR = HID // P          # 4  row-blocks of H per gate
NM = 4 * HID // P      # 16 gate-row blocks total (i,g,f,o order)

# gate burst order per group: f first (longest elementwise suffix),
# o last (shortest suffix: just ACT + mul + dma)
GATE_ORDER = (2, 0, 1, 3)  # f, i, g, o   (gate index: 0=i 1=g 2=f 3=o)


def build_nc(bl=BL):
    """Build the single-core Bass program (SPMD-replicated across cores)."""
    nbn = bl // JW
    nc = bacc.Bacc("TRN2", target_bir_lowering=False, debug=False)

    # xh: per batch-chunk nb, partition p holds row k*128+p of [x;h].T for
    # all k, contiguous: [nb][p][k*JW + jw]
    xh_in = nc.dram_tensor("xh_in", [nbn, P, NK * JW], BF16, kind="ExternalInput")
    # w: contiguous 256KB chunk per (k, rhalf): [k][rh][p][(g*2+rl)*128+j]
    # holding W_cat.T[k*128+p, m*128+j] for m = g*NR + (rh*2+rl). 2KB lines:
    # same line size as the xh0 chunks, so DMA shares stay fair early on.
    wt_in = nc.dram_tensor("wt_in", [NK, 2, P, 8 * P], BF16, kind="ExternalInput")
    # bias replicated 32x along free dim: 2KB per-partition lines. DMA
    # bandwidth share is proportional to line size, so a 64B-line transfer
    # would starve its whole queue for ~8us (measured).
    bias_in = nc.dram_tensor("bias_in", [P, 32 * NM], F32, kind="ExternalInput")
    c_in = nc.dram_tensor("c_in", [nbn, P, NR * JW], F32, kind="ExternalInput")
    h_out = nc.dram_tensor("h_out", [nbn, P, NR * JW], F32, kind="ExternalOutput")
    c_out = nc.dram_tensor("c_out", [nbn, P, NR * JW], F32, kind="ExternalOutput")

    with ExitStack() as ctx:
        tc = ctx.enter_context(tile.TileContext(nc))
        wpool = ctx.enter_context(tc.tile_pool(name="w", bufs=1))
        xpool = ctx.enter_context(tc.tile_pool(name="xh", bufs=1))
        cpool = ctx.enter_context(tc.tile_pool(name="cin", bufs=1))
        gpool = ctx.enter_context(tc.tile_pool(name="gates", bufs=3))
        opool = ctx.enter_context(tc.tile_pool(name="outs", bufs=3))
        pspool = ctx.enter_context(tc.tile_pool(name="ps", bufs=1, space="PSUM"))

        ps = [
            pspool.tile([P, JW], F32, tag=f"p{i}", name=f"p{i}") for i in range(8)
        ]

        # --- warmup: keep the PE activity monitor busy from preamble end so
        # the clock is at 2.4GHz when real matmuls start. Depends only on a
        # tiny memset, not on any DMA.
        wu = wpool.tile([P, P], BF16, tag="wu", name="wu")
        nc.vector.memset(wu[:], 0.0)
        # ~48 cold N=128 MMs ~= 5.1us busy: bridges from preamble end until
        # the first data chunks land (~13us) with the HAM already at 8/8.
        for _ in range(48):
            nc.tensor.matmul(ps[7][:, :P], wu[:], wu[:], start=True, stop=True)

        # --- SBUF tiles. xh chunk 0 is split (k0-1 / k2-7) and c chunks are
        # split in r-halves so early consumers don't wait on whole tiles.
        xh0a = xpool.tile([P, 2 * JW], BF16, tag="xh0a", name="xh0a")
        xh0b = xpool.tile([P, 2 * JW], BF16, tag="xh0b", name="xh0b")
        xh0c = xpool.tile([P, 4 * JW], BF16, tag="xh0c", name="xh0c")
        xh_t = [None] + [
            xpool.tile([P, NK * JW], BF16, tag=f"xh{nb}", name=f"xh{nb}")
            for nb in range(1, nbn)
        ]
        c_t = [
            [
                cpool.tile([P, 2 * JW], F32, tag=f"c{nb}_{rh}", name=f"c{nb}_{rh}")
                for rh in range(2)
            ]
            for nb in range(nbn)
        ]
        bias_t = wpool.tile([P, 32 * NM], F32, tag="bias", name="bias")

        def xh_sl(nb, k):
            if nb == 0:
                t, o = (xh0a, 0) if k < 2 else (xh0b, 2) if k < 4 else (xh0c, 4)
                return t[:, (k - o) * JW : (k - o + 1) * JW]
            return xh_t[nb][:, k * JW : (k + 1) * JW]

        # --- DMA issue plan (queue order = priority; lines are 2-8KB).
        # Critical window carries ONLY phase A's diet: xh0+bias+c0a on the
        # scalar queue, weight k-chunks alternating sync/gpsimd (a double
        # bandwidth share). Everything else queues behind the weights
        # (in-order drain = free prioritization).
        nc.scalar.dma_start(xh0a[:], xh_in[0][:, : 2 * JW])
        nc.scalar.dma_start(bias_t[:], bias_in[:])
        nc.scalar.dma_start(xh0b[:], xh_in[0][:, 2 * JW : 4 * JW])
        nc.scalar.dma_start(xh0c[:], xh_in[0][:, 4 * JW :])
        nc.scalar.dma_start(c_t[0][0][:], c_in[0][:, : 2 * JW])
        wts = [[None] * 2 for _ in range(NK)]
        for rh in range(2):
            for k in range(NK):
                wt = wpool.tile(
                    [P, 8 * P], BF16, tag=f"w{k}_{rh}", name=f"w{k}_{rh}"
                )
                wts[k][rh] = wt
                eng = nc.sync if k % 2 == 0 else nc.gpsimd
                eng.dma_start(wt[:], wt_in[k, rh])
        # late inputs behind the weights, earliest-needed first
        late = [(nc.sync, c_t[0][1], c_in[0][:, 2 * JW :])]
        for nb in range(1, nbn):
            e1, e2 = (nc.gpsimd, nc.sync) if nb % 2 else (nc.sync, nc.gpsimd)
            late.append((e1, xh_t[nb], xh_in[nb]))
            late.append((e2, c_t[nb][0], c_in[nb][:, : 2 * JW]))
            late.append((e1, c_t[nb][1], c_in[nb][:, 2 * JW :]))
        for eng, dst, src in late:
            eng.dma_start(dst[:], src)

        def mm_burst(nb, r, gg, base):
            """8 accumulating matmuls (k-inner) for gate gg of (nb, r)."""
            rh, rl = r // 2, r % 2
            col = (gg * 2 + rl) * P
            for k in range(NK):
                nc.tensor.matmul(
                    ps[base + gg][:],
                    wts[k][rh][:, col : col + P],
                    xh_sl(nb, k),
                    start=(k == 0),
                    stop=(k == NK - 1),
                )

        def mm_group_kouter(nb, r, base):
            """k-outer group: consumes weight chunks in arrival order."""
            rh, rl = r // 2, r % 2
            for k in range(NK):
                for gg in GATE_ORDER:
                    col = (gg * 2 + rl) * P
                    nc.tensor.matmul(
                        ps[base + gg][:],
                        wts[k][rh][:, col : col + P],
                        xh_sl(nb, k),
                        start=(k == 0),
                        stop=(k == NK - 1),
                    )

        def mm_rpair_kouter(nb, rh):
            """k-outer phase over r-pair (2rh, 2rh+1) across all 8 PSUM
            banks: consumes weight chunks in arrival order. Gate g of
            r-local rl -> bank rl*4+g."""
            for k in range(NK):
                for gg in GATE_ORDER:
                    for rl in range(2):
                        col = (gg * 2 + rl) * P
                        nc.tensor.matmul(
                            ps[rl * 4 + gg][:],
                            wts[k][rh][:, col : col + P],
                            xh_sl(nb, k),
                            start=(k == 0),
                            stop=(k == NK - 1),
                        )

        def act_gate(nb, r, gg, base, sl, dst):
            fn = AF.Tanh if gg == 1 else AF.Sigmoid
            nc.scalar.activation(
                dst[:, sl], ps[base + gg][:, sl], fn,
                bias=bias_t[:, gg * NR + r : gg * NR + r + 1],
            )

        def elementwise(nb, r, base, split=1):
            """Cell update for group (nb, r); gates in ps[base..base+3].
            ACT eviction order f,i,g,o matches the bank-demand order of the
            next group using these banks (its bursts are f,i,g,o too)."""
            cw = JW // split
            for s in range(split):
                sl = slice(s * cw, (s + 1) * cw)
                csl = slice((r % 2) * JW + s * cw, (r % 2) * JW + (s + 1) * cw)
                cti = c_t[nb][r // 2]
                ft = gpool.tile([P, JW], F32, tag="f")
                it = gpool.tile([P, JW], F32, tag="i")
                gt = gpool.tile([P, JW], F32, tag="g")
                ot = gpool.tile([P, JW], F32, tag="o")
                t1 = gpool.tile([P, JW], F32, tag="t1")
                t2 = gpool.tile([P, JW], F32, tag="t2")
                tch = gpool.tile([P, JW], F32, tag="tch")
                cn = opool.tile([P, JW], F32, tag="cn")
                hn = opool.tile([P, JW], F32, tag="hn")
                osl = slice(r * JW + s * cw, r * JW + (s + 1) * cw)
                act_gate(nb, r, 2, base, sl, ft)
                nc.vector.tensor_mul(t2[:, sl], ft[:, sl], cti[:, csl])
                act_gate(nb, r, 0, base, sl, it)
                act_gate(nb, r, 1, base, sl, gt)
                nc.vector.tensor_mul(t1[:, sl], it[:, sl], gt[:, sl])
                nc.vector.tensor_add(cn[:, sl], t1[:, sl], t2[:, sl])
                nc.scalar.activation(tch[:, sl], cn[:, sl], AF.Tanh)
                nc.sync.dma_start(c_out[nb][:, osl], cn[:, sl])
                act_gate(nb, r, 3, base, sl, ot)
                nc.vector.tensor_mul(hn[:, sl], ot[:, sl], tch[:, sl])
                nc.sync.dma_start(h_out[nb][:, osl], hn[:, sl])

        # --- phase A: batch-chunk 0 entirely k-outer, consuming weight
        # chunks in arrival order. First the r0+r1 pair across all 8 banks,
        # then r2 alone (r0's banks, freed gate-by-gate in matching order)
        # and r3 alone (r1's banks, freed long before).
        mm_rpair_kouter(0, 0)
        elementwise(0, 0, base=0)
        mm_group_kouter(0, 2, base=0)
        elementwise(0, 1, base=4)
        elementwise(0, 2, base=0)
        mm_group_kouter(0, 3, base=4)
        elementwise(0, 3, base=4)

        # --- steady state: remaining groups, g-outer k-inner, alternating
        # PSUM halves for double buffering.
        groups = [
            (nb, r) for nb in range(1, nbn) for r in range(NR)
        ]
        for j, (nb, r) in enumerate(groups):
            base = 4 * (j % 2)
            for gg in GATE_ORDER:
                mm_burst(nb, r, gg, base)
            last = j == len(groups) - 1
            elementwise(nb, r, base, split=2 if last else 1)

    nc.compile()
    return nc


def prep_shared(Wxi, Wxg, Wxf, Wxo, Whi, Whg, Whf, Who, bias_sum):
    """wt_in [NK,P,4H] bf16 and bias_in [P,NM] f32 (gate order i,g,f,o)."""
    Wx = np.concatenate([Wxi, Wxg, Wxf, Wxo], axis=0)  # [4H, IN]
    Wh = np.concatenate([Whi, Whg, Whf, Who], axis=0)  # [4H, HID]
    WT = np.concatenate([Wx.T, Wh.T], axis=0)          # [K=1024, 4H]
    # WT[k*128+p, m*128+j] -> wt[k, rh, p, (g*2+rl)*128+j], m = g*NR+rh*2+rl
    W6 = WT.reshape(NK, P, 4, 2, 2, P)        # [k, p, g, rh, rl, j]
    wt_arr = np.ascontiguousarray(
        W6.transpose(0, 3, 1, 2, 4, 5)        # [k, rh, p, g, rl, j]
        .reshape(NK, 2, P, 8 * P)
        .astype(BF16_NP)
    )
    bias_arr = np.ascontiguousarray(
        np.tile(bias_sum.reshape(NM, P).T.astype(np.float32), (1, 32))
    )
    return wt_arr, bias_arr


def prep_core(x_s, h_s, c_s):
    """Per-core xh_in [nb,P,NK*JW] bf16 and c_in [nb,P,NR*JW] f32."""
    bl = x_s.shape[0]
    nbn = bl // JW
    xhT = np.concatenate([x_s, h_s], axis=1).T  # [K=1024, bl]
    # xhT[k*128+p, nb*JW+jw] -> xh[nb, p, k*JW+jw]
    xh_arr = np.ascontiguousarray(
        xhT.reshape(NK, P, nbn, JW).transpose(2, 1, 0, 3)
        .reshape(nbn, P, NK * JW)
        .astype(BF16_NP)
    )
    cT = c_s.T  # [HID, bl]
    c_arr = np.ascontiguousarray(
        cT.reshape(NR, P, nbn, JW).transpose(2, 1, 0, 3)
        .reshape(nbn, P, NR * JW)
        .astype(np.float32)
    )
    return xh_arr, c_arr


def post_core(arr):
    """[nb,P,NR*JW] -> [bl, HID]"""
    arr = np.asarray(arr)
    nbn = arr.size // (NR * P * JW)
    arr = arr.reshape(nbn, P, NR, JW)
    return arr.transpose(0, 3, 2, 1).reshape(nbn * JW, HID)


_NC_CACHE = {}


def _get_nc(bl=BL):
    if bl not in _NC_CACHE:
        _NC_CACHE[bl] = build_nc(bl)
    return _NC_CACHE[bl]


def make_in_maps(x, h, c, Wxi, bxi, Wxo, bxo, Wxf, bxf, Wxg, bxg,
                 Whi, bhi, Who, bho, Whf, bhf, Whg, bhg, ncores=NCORES):
    bias_sum = np.concatenate(
        [bxi + bhi, bxg + bhg, bxf + bhf, bxo + bho], axis=0
    ).astype(np.float32)
    wt_arr, bias_arr = prep_shared(Wxi, Wxg, Wxf, Wxo, Whi, Whg, Whf, Who, bias_sum)
    bl = x.shape[0] // ncores
    in_maps = []
    for i in range(ncores):
        s = slice(i * bl, (i + 1) * bl)
        xh_arr, c_arr = prep_core(
            np.asarray(x[s], np.float32),
            np.asarray(h[s], np.float32),
            np.asarray(c[s], np.float32),
        )
        in_maps.append(
            {"xh_in": xh_arr, "wt_in": wt_arr, "bias_in": bias_arr, "c_in": c_arr}
        )
    return in_maps


def kernel(x, h, c, Wxi, bxi, Wxo, bxo, Wxf, bxf, Wxg, bxg,
           Whi, bhi, Who, bho, Whf, bhf, Whg, bhg):
    args = dict(
        x=np.asarray(x, np.float32), h=np.asarray(h, np.float32),
        c=np.asarray(c, np.float32),
        Wxi=np.asarray(Wxi, np.float32), bxi=np.asarray(bxi, np.float32),
        Wxo=np.asarray(Wxo, np.float32), bxo=np.asarray(bxo, np.float32),
        Wxf=np.asarray(Wxf, np.float32), bxf=np.asarray(bxf, np.float32),
        Wxg=np.asarray(Wxg, np.float32), bxg=np.asarray(bxg, np.float32),
        Whi=np.asarray(Whi, np.float32), bhi=np.asarray(bhi, np.float32),
        Who=np.asarray(Who, np.float32), bho=np.asarray(bho, np.float32),
        Whf=np.asarray(Whf, np.float32), bhf=np.asarray(bhf, np.float32),
        Whg=np.asarray(Whg, np.float32), bhg=np.asarray(bhg, np.float32),
    )
    in_maps = make_in_maps(**args)
    nc = _get_nc(BL)
    res = run_bass_kernel_spmd(nc, in_maps, core_ids=list(range(NCORES)))
    h_new = np.empty((B_FULL, HID), np.float32)
    c_new = np.empty((B_FULL, HID), np.float32)
    for i in range(NCORES):
        s = slice(i * BL, (i + 1) * BL)
        h_new[s] = post_core(res.results[i]["h_out"])
        c_new[s] = post_core(res.results[i]["c_out"])
    return (h_new, c_new)


# revision 18
# speedup vs baseline: 1.0153x; 1.0153x over previous
"""LSTMCell (B=16384, IN=HID=512) on 8 TRN2 NeuronCores.

Strategy: data-parallel over batch (2048 rows/core), weights replicated.
Host pre-packs operands so the device kernel needs zero transposes:
  - GEMM computed as gates.T = W_cat.T @ [x;h].T  (K=1024 on partitions)
  - x/h/W cast to bf16 on host (fp32 PSUM accumulation on PE)
  - c / outputs stay fp32

Schedule (v4): the PE floor for this GEMM is 512 MMs x 512 cols ~= 109us
at bf16; the wins over the v1 baseline are at the edges:
  - batch-chunk 0 (r0+r1) runs K-OUTER across all 8 PSUM banks,
    consuming 512KB weight k-chunks as they stream in, so real matmuls
    start once w[k=0] + xh0[k<2] land instead of waiting for the full
    4MB weight load.
  - DMA bandwidth share is proportional to per-partition line size and
    queues drain in order, so: every bulk tensor keeps 4-8KB lines, the
    weight chunks alternate across the sync and gpsimd queues (double
    share during the critical window), and late-use inputs are queued
    BEHIND the weights on the same queues instead of contending.
  - warmup MMs depend only on a tiny memset, covering the HAM cold-clock
    window during the first DMAs without delaying real work.
  - per-group gate order (f,i,g,o) with eagerly emitted ACT/DVE chain:
    eviction order matches the next group's bank-demand order, and the
    end-of-kernel tail shrinks from ~6.7us to ~2us.
"""

import sys

sys.path.insert(0, "/opt/trn_rl_repo")

from contextlib import ExitStack

import ml_dtypes
import numpy as np

import concourse.bass as bass  # noqa: F401  (bass types used via bacc/mybir)
import concourse.mybir as mybir
import concourse.tile as tile
from concourse import bacc
from concourse.bass_utils import run_bass_kernel_spmd

B_FULL, IN, HID = 16384, 512, 512
NCORES = 8
BL = B_FULL // NCORES  # 2048 batch rows per core
JW = 512               # batch columns per chunk (matmul free dim)
P = 128

BF16 = mybir.dt.bfloat16
F32 = mybir.dt.float32
AF = mybir.ActivationFunctionType
BF16_NP = ml_dtypes.bfloat16

NK = (IN + HID) // P   # 8  k-chunks of the contraction dim
NR = HID // P          # 4  row-blocks of H per gate
NM = 4 * HID // P      # 16 gate-row blocks total (i,g,f,o order)

# gate burst order per group: f first (longest elementwise suffix),
# o last (shortest suffix: just ACT + mul + dma)
GATE_ORDER = (2, 0, 1, 3)  # f, i, g, o   (gate index: 0=i 1=g 2=f 3=o)


def build_nc(bl=BL):
    """Build the single-core Bass program (SPMD-replicated across cores)."""
    nbn = bl // JW
    nc = bacc.Bacc("TRN2", target_bir_lowering=False, debug=False)

    # xh: per batch-chunk nb, partition p holds row k*128+p of [x;h].T for
    # all k, contiguous: [nb][p][k*JW + jw]
    xh_in = nc.dram_tensor("xh_in", [nbn, P, NK * JW], BF16, kind="ExternalInput")
    # w: contiguous 256KB chunk per (k, rhalf): [k][rh][p][(g*2+rl)*128+j]
    # holding W_cat.T[k*128+p, m*128+j] for m = g*NR + (rh*2+rl). 2KB lines:
    # same line size as the xh0 chunks, so DMA shares stay fair early on.
    wt_in = nc.dram_tensor("wt_in", [NK, 2, P, 8 * P], BF16, kind="ExternalInput")
    # bias replicated 32x along free dim: 2KB per-partition lines. DMA
    # bandwidth share is proportional to line size, so a 64B-line transfer
    # would starve its whole queue for ~8us (measured).
    bias_in = nc.dram_tensor("bias_in", [P, 32 * NM], F32, kind="ExternalInput")
    c_in = nc.dram_tensor("c_in", [nbn, P, NR * JW], F32, kind="ExternalInput")
    h_out = nc.dram_tensor("h_out", [nbn, P, NR * JW], F32, kind="ExternalOutput")
    c_out = nc.dram_tensor("c_out", [nbn, P, NR * JW], F32, kind="ExternalOutput")

    with ExitStack() as ctx:
        tc = ctx.enter_context(tile.TileContext(nc))
        wpool = ctx.enter_context(tc.tile_pool(name="w", bufs=1))
        xpool = ctx.enter_context(tc.tile_pool(name="xh", bufs=1))
        cpool = ctx.enter_context(tc.tile_pool(name="cin", bufs=1))
        gpool = ctx.enter_context(tc.tile_pool(name="gates", bufs=3))
        opool = ctx.enter_context(tc.tile_pool(name="outs", bufs=3))
        pspool = ctx.enter_context(tc.tile_pool(name="ps", bufs=1, space="PSUM"))

        ps = [
            pspool.tile([P, JW], F32, tag=f"p{i}", name=f"p{i}") for i in range(8)
        ]

        # --- warmup: keep the PE activity monitor busy from preamble end so
        # the clock is at 2.4GHz when real matmuls start. Depends only on a
        # tiny memset, not on any DMA.
        wu = wpool.tile([P, P], BF16, tag="wu", name="wu")
        nc.vector.memset(wu[:], 0.0)
        # ~48 cold N=128 MMs ~= 5.1us busy: bridges from preamble end until
        # the first data chunks land (~13us) with the HAM already at 8/8.
        for _ in range(48):
            nc.tensor.matmul(ps[7][:, :P], wu[:], wu[:], start=True, stop=True)

        # --- SBUF tiles. xh chunk 0 is split (k0-1 / k2-7) and c chunks are
        # split in r-halves so early consumers don't wait on whole tiles.
        xh0a = xpool.tile([P, 2 * JW], BF16, tag="xh0a", name="xh0a")
        xh0b = xpool.tile([P, 2 * JW], BF16, tag="xh0b", name="xh0b")
        xh0c = xpool.tile([P, 4 * JW], BF16, tag="xh0c", name="xh0c")
        xh_t = [None] + [
            xpool.tile([P, NK * JW], BF16, tag=f"xh{nb}", name=f"xh{nb}")
            for nb in range(1, nbn)
        ]
        c_t = [
            [
                cpool.tile([P, 2 * JW], F32, tag=f"c{nb}_{rh}", name=f"c{nb}_{rh}")
                for rh in range(2)
            ]
            for nb in range(nbn)
        ]
        bias_t = wpool.tile([P, 32 * NM], F32, tag="bias", name="bias")

        def xh_sl(nb, k):
            if nb == 0:
                t, o = (xh0a, 0) if k < 2 else (xh0b, 2) if k < 4 else (xh0c, 4)
                return t[:, (k - o) * JW : (k - o + 1) * JW]
            return xh_t[nb][:, k * JW : (k + 1) * JW]

        # --- DMA issue plan (queue order = priority; lines are 2-8KB).
        # Critical window carries ONLY phase A's diet: xh0+bias+c0a on the
        # scalar queue, weight k-chunks alternating sync/gpsimd (a double
        # bandwidth share). Everything else queues behind the weights
        # (in-order drain = free prioritization).
        nc.scalar.dma_start(xh0a[:], xh_in[0][:, : 2 * JW])
        nc.scalar.dma_start(xh0b[:], xh_in[0][:, 2 * JW : 4 * JW])
        nc.scalar.dma_start(xh0c[:], xh_in[0][:, 4 * JW :])
        nc.scalar.dma_start(bias_t[:], bias_in[:])
        nc.scalar.dma_start(c_t[0][0][:], c_in[0][:, : 2 * JW])
        wts = [[None] * 2 for _ in range(NK)]
        for rh in range(2):
            for k in range(NK):
                wt = wpool.tile(
                    [P, 8 * P], BF16, tag=f"w{k}_{rh}", name=f"w{k}_{rh}"
                )
                wts[k][rh] = wt
                eng = nc.sync if k % 2 == 0 else nc.gpsimd
                eng.dma_start(wt[:], wt_in[k, rh])
        # late inputs behind the weights, earliest-needed first
        late = [(nc.sync, c_t[0][1], c_in[0][:, 2 * JW :])]
        for nb in range(1, nbn):
            e1, e2 = (nc.gpsimd, nc.sync) if nb % 2 else (nc.sync, nc.gpsimd)
            late.append((e1, xh_t[nb], xh_in[nb]))
            late.append((e2, c_t[nb][0], c_in[nb][:, : 2 * JW]))
            late.append((e1, c_t[nb][1], c_in[nb][:, 2 * JW :]))
        for eng, dst, src in late:
            eng.dma_start(dst[:], src)

        def mm_burst(nb, r, gg, base):
            """8 accumulating matmuls (k-inner) for gate gg of (nb, r)."""
            rh, rl = r // 2, r % 2
            col = (gg * 2 + rl) * P
            for k in range(NK):
                nc.tensor.matmul(
                    ps[base + gg][:],
                    wts[k][rh][:, col : col + P],
                    xh_sl(nb, k),
                    start=(k == 0),
                    stop=(k == NK - 1),
                )

        def mm_group_kouter(nb, r, base):
            """k-outer group: consumes weight chunks in arrival order."""
            rh, rl = r // 2, r % 2
            for k in range(NK):
                for gg in GATE_ORDER:
                    col = (gg * 2 + rl) * P
                    nc.tensor.matmul(
                        ps[base + gg][:],
                        wts[k][rh][:, col : col + P],
                        xh_sl(nb, k),
                        start=(k == 0),
                        stop=(k == NK - 1),
                    )

        def mm_rpair_kouter(nb, rh):
            """k-outer phase over r-pair (2rh, 2rh+1) across all 8 PSUM
            banks: consumes weight chunks in arrival order. Gate g of
            r-local rl -> bank rl*4+g."""
            for k in range(NK):
                for gg in GATE_ORDER:
                    for rl in range(2):
                        col = (gg * 2 + rl) * P
                        nc.tensor.matmul(
                            ps[rl * 4 + gg][:],
                            wts[k][rh][:, col : col + P],
                            xh_sl(nb, k),
                            start=(k == 0),
                            stop=(k == NK - 1),
                        )

        def act_gate(nb, r, gg, base, sl, dst):
            fn = AF.Tanh if gg == 1 else AF.Sigmoid
            nc.scalar.activation(
                dst[:, sl], ps[base + gg][:, sl], fn,
                bias=bias_t[:, gg * NR + r : gg * NR + r + 1],
            )

        def elementwise(nb, r, base, split=1):
            """Cell update for group (nb, r); gates in ps[base..base+3].
            ACT eviction order f,i,g,o matches the bank-demand order of the
            next group using these banks (its bursts are f,i,g,o too)."""
            cw = JW // split
            for s in range(split):
                sl = slice(s * cw, (s + 1) * cw)
                csl = slice((r % 2) * JW + s * cw, (r % 2) * JW + (s + 1) * cw)
                cti = c_t[nb][r // 2]
                ft = gpool.tile([P, JW], F32, tag="f")
                it = gpool.tile([P, JW], F32, tag="i")
                gt = gpool.tile([P, JW], F32, tag="g")
                ot = gpool.tile([P, JW], F32, tag="o")
                t1 = gpool.tile([P, JW], F32, tag="t1")
                t2 = gpool.tile([P, JW], F32, tag="t2")
                tch = gpool.tile([P, JW], F32, tag="tch")
                cn = opool.tile([P, JW], F32, tag="cn")
                hn = opool.tile([P, JW], F32, tag="hn")
                osl = slice(r * JW + s * cw, r * JW + (s + 1) * cw)
                act_gate(nb, r, 2, base, sl, ft)
                nc.vector.tensor_mul(t2[:, sl], ft[:, sl], cti[:, csl])
                act_gate(nb, r, 0, base, sl, it)
                act_gate(nb, r, 1, base, sl, gt)
                nc.vector.tensor_mul(t1[:, sl], it[:, sl], gt[:, sl])
                nc.vector.tensor_add(cn[:, sl], t1[:, sl], t2[:, sl])
                nc.scalar.activation(tch[:, sl], cn[:, sl], AF.Tanh)
                nc.sync.dma_start(c_out[nb][:, osl], cn[:, sl])
                act_gate(nb, r, 3, base, sl, ot)
                nc.vector.tensor_mul(hn[:, sl], ot[:, sl], tch[:, sl])
                nc.sync.dma_start(h_out[nb][:, osl], hn[:, sl])

        # --- phase A: batch-chunk 0 entirely k-outer, consuming weight
        # chunks in arrival order. First the r0+r1 pair across all 8 banks,
        # then r2 alone (r0's banks, freed gate-by-gate in matching order)
        # and r3 alone (r1's banks, freed long before).
        mm_rpair_kouter(0, 0)
        elementwise(0, 0, base=0)
        mm_group_kouter(0, 2, base=0)
        elementwise(0, 1, base=4)
        elementwise(0, 2, base=0)
        mm_group_kouter(0, 3, base=4)
        elementwise(0, 3, base=4)

        # --- steady state: remaining groups, g-outer k-inner, alternating
        # PSUM halves for double buffering.
        groups = [
            (nb, r) for nb in range(1, nbn) for r in range(NR)
        ]
        for j, (nb, r) in enumerate(groups):
            base = 4 * (j % 2)
            for gg in GATE_ORDER:
                mm_burst(nb, r, gg, base)
            last = j == len(groups) - 1
            elementwise(nb, r, base, split=2 if last else 1)

    nc.compile()
    return nc


def prep_shared(Wxi, Wxg, Wxf, Wxo, Whi, Whg, Whf, Who, bias_sum):
    """wt_in [NK,P,4H] bf16 and bias_in [P,NM] f32 (gate order i,g,f,o)."""
    Wx = np.concatenate([Wxi, Wxg, Wxf, Wxo], axis=0)  # [4H, IN]
    Wh = np.concatenate([Whi, Whg, Whf, Who], axis=0)  # [4H, HID]
    WT = np.concatenate([Wx.T, Wh.T], axis=0)          # [K=1024, 4H]
    # WT[k*128+p, m*128+j] -> wt[k, rh, p, (g*2+rl)*128+j], m = g*NR+rh*2+rl
    W6 = WT.reshape(NK, P, 4, 2, 2, P)        # [k, p, g, rh, rl, j]
    wt_arr = np.ascontiguousarray(
        W6.transpose(0, 3, 1, 2, 4, 5)        # [k, rh, p, g, rl, j]
        .reshape(NK, 2, P, 8 * P)
        .astype(BF16_NP)
    )
    bias_arr = np.ascontiguousarray(
        np.tile(bias_sum.reshape(NM, P).T.astype(np.float32), (1, 32))
    )
    return wt_arr, bias_arr


def prep_core(x_s, h_s, c_s):
    """Per-core xh_in [nb,P,NK*JW] bf16 and c_in [nb,P,NR*JW] f32."""
    bl = x_s.shape[0]
    nbn = bl // JW
    xhT = np.concatenate([x_s, h_s], axis=1).T  # [K=1024, bl]
    # xhT[k*128+p, nb*JW+jw] -> xh[nb, p, k*JW+jw]
    xh_arr = np.ascontiguousarray(
        xhT.reshape(NK, P, nbn, JW).transpose(2, 1, 0, 3)
        .reshape(nbn, P, NK * JW)
        .astype(BF16_NP)
    )
    cT = c_s.T  # [HID, bl]
    c_arr = np.ascontiguousarray(
        cT.reshape(NR, P, nbn, JW).transpose(2, 1, 0, 3)
        .reshape(nbn, P, NR * JW)
        .astype(np.float32)
    )
    return xh_arr, c_arr


def post_core(arr):
    """[nb,P,NR*JW] -> [bl, HID]"""
    arr = np.asarray(arr)
    nbn = arr.size // (NR * P * JW)
    arr = arr.reshape(nbn, P, NR, JW)
    return arr.transpose(0, 3, 2, 1).reshape(nbn * JW, HID)


_NC_CACHE = {}


def _get_nc(bl=BL):
    if bl not in _NC_CACHE:
        _NC_CACHE[bl] = build_nc(bl)
    return _NC_CACHE[bl]


def make_in_maps(x, h, c, Wxi, bxi, Wxo, bxo, Wxf, bxf, Wxg, bxg,
                 Whi, bhi, Who, bho, Whf, bhf, Whg, bhg, ncores=NCORES):
    bias_sum = np.concatenate(
        [bxi + bhi, bxg + bhg, bxf + bhf, bxo + bho], axis=0
    ).astype(np.float32)
    wt_arr, bias_arr = prep_shared(Wxi, Wxg, Wxf, Wxo, Whi, Whg, Whf, Who, bias_sum)
    bl = x.shape[0] // ncores
    in_maps = []
    for i in range(ncores):
        s = slice(i * bl, (i + 1) * bl)
        xh_arr, c_arr = prep_core(
            np.asarray(x[s], np.float32),
            np.asarray(h[s], np.float32),
            np.asarray(c[s], np.float32),
        )
        in_maps.append(
            {"xh_in": xh_arr, "wt_in": wt_arr, "bias_in": bias_arr, "c_in": c_arr}
        )
    return in_maps


def kernel(x, h, c, Wxi, bxi, Wxo, bxo, Wxf, bxf, Wxg, bxg,
           Whi, bhi, Who, bho, Whf, bhf, Whg, bhg):
    args = dict(
        x=np.asarray(x, np.float32), h=np.asarray(h, np.float32),
        c=np.asarray(c, np.float32),
        Wxi=np.asarray(Wxi, np.float32), bxi=np.asarray(bxi, np.float32),
        Wxo=np.asarray(Wxo, np.float32), bxo=np.asarray(bxo, np.float32),
        Wxf=np.asarray(Wxf, np.float32), bxf=np.asarray(bxf, np.float32),
        Wxg=np.asarray(Wxg, np.float32), bxg=np.asarray(bxg, np.float32),
        Whi=np.asarray(Whi, np.float32), bhi=np.asarray(bhi, np.float32),
        Who=np.asarray(Who, np.float32), bho=np.asarray(bho, np.float32),
        Whf=np.asarray(Whf, np.float32), bhf=np.asarray(bhf, np.float32),
        Whg=np.asarray(Whg, np.float32), bhg=np.asarray(bhg, np.float32),
    )
    in_maps = make_in_maps(**args)
    nc = _get_nc(BL)
    res = run_bass_kernel_spmd(nc, in_maps, core_ids=list(range(NCORES)))
    h_new = np.empty((B_FULL, HID), np.float32)
    c_new = np.empty((B_FULL, HID), np.float32)
    for i in range(NCORES):
        s = slice(i * BL, (i + 1) * BL)
        h_new[s] = post_core(res.results[i]["h_out"])
        c_new[s] = post_core(res.results[i]["c_out"])
    return (h_new, c_new)


# revision 19
# speedup vs baseline: 1.0182x; 1.0028x over previous
"""LSTMCell (B=16384, IN=HID=512) on 8 TRN2 NeuronCores.

Strategy: data-parallel over batch (2048 rows/core), weights replicated.
Host pre-packs operands so the device kernel needs zero transposes:
  - GEMM computed as gates.T = W_cat.T @ [x;h].T  (K=1024 on partitions)
  - x/h/W cast to bf16 on host (fp32 PSUM accumulation on PE)
  - c / outputs stay fp32

Schedule (v4): the PE floor for this GEMM is 512 MMs x 512 cols ~= 109us
at bf16; the wins over the v1 baseline are at the edges:
  - batch-chunk 0 (r0+r1) runs K-OUTER across all 8 PSUM banks,
    consuming 512KB weight k-chunks as they stream in, so real matmuls
    start once w[k=0] + xh0[k<2] land instead of waiting for the full
    4MB weight load.
  - DMA bandwidth share is proportional to per-partition line size and
    queues drain in order, so: every bulk tensor keeps 4-8KB lines, the
    weight chunks alternate across the sync and gpsimd queues (double
    share during the critical window), and late-use inputs are queued
    BEHIND the weights on the same queues instead of contending.
  - warmup MMs depend only on a tiny memset, covering the HAM cold-clock
    window during the first DMAs without delaying real work.
  - per-group gate order (f,i,g,o) with eagerly emitted ACT/DVE chain:
    eviction order matches the next group's bank-demand order, and the
    end-of-kernel tail shrinks from ~6.7us to ~2us.
"""

import sys

sys.path.insert(0, "/opt/trn_rl_repo")

from contextlib import ExitStack

import ml_dtypes
import numpy as np

import concourse.bass as bass  # noqa: F401  (bass types used via bacc/mybir)
import concourse.mybir as mybir
import concourse.tile as tile
from concourse import bacc
from concourse.bass_utils import run_bass_kernel_spmd

B_FULL, IN, HID = 16384, 512, 512
NCORES = 8
BL = B_FULL // NCORES  # 2048 batch rows per core
JW = 512               # batch columns per chunk (matmul free dim)
P = 128

BF16 = mybir.dt.bfloat16
F32 = mybir.dt.float32
AF = mybir.ActivationFunctionType
BF16_NP = ml_dtypes.bfloat16

NK = (IN + HID) // P   # 8  k-chunks of the contraction dim
NR = HID // P          # 4  row-blocks of H per gate
NM = 4 * HID // P      # 16 gate-row blocks total (i,g,f,o order)

# gate burst order per group: f first (longest elementwise suffix),
# o last (shortest suffix: just ACT + mul + dma)
GATE_ORDER = (2, 0, 1, 3)  # f, i, g, o   (gate index: 0=i 1=g 2=f 3=o)


def build_nc(bl=BL):
    """Build the single-core Bass program (SPMD-replicated across cores)."""
    nbn = bl // JW
    nc = bacc.Bacc("TRN2", target_bir_lowering=False, debug=False)

    # xh: per batch-chunk nb, partition p holds row k*128+p of [x;h].T for
    # all k, contiguous: [nb][p][k*JW + jw]
    xh_in = nc.dram_tensor("xh_in", [nbn, P, NK * JW], BF16, kind="ExternalInput")
    # w: contiguous 256KB chunk per (k, rhalf): [k][rh][p][(g*2+rl)*128+j]
    # holding W_cat.T[k*128+p, m*128+j] for m = g*NR + (rh*2+rl). 2KB lines:
    # same line size as the xh0 chunks, so DMA shares stay fair early on.
    wt_in = nc.dram_tensor("wt_in", [NK, 2, P, 8 * P], BF16, kind="ExternalInput")
    # bias replicated 32x along free dim: 2KB per-partition lines. DMA
    # bandwidth share is proportional to line size, so a 64B-line transfer
    # would starve its whole queue for ~8us (measured).
    bias_in = nc.dram_tensor("bias_in", [P, 32 * NM], F32, kind="ExternalInput")
    c_in = nc.dram_tensor("c_in", [nbn, P, NR * JW], F32, kind="ExternalInput")
    h_out = nc.dram_tensor("h_out", [nbn, P, NR * JW], F32, kind="ExternalOutput")
    c_out = nc.dram_tensor("c_out", [nbn, P, NR * JW], F32, kind="ExternalOutput")

    with ExitStack() as ctx:
        tc = ctx.enter_context(tile.TileContext(nc))
        wpool = ctx.enter_context(tc.tile_pool(name="w", bufs=1))
        xpool = ctx.enter_context(tc.tile_pool(name="xh", bufs=1))
        cpool = ctx.enter_context(tc.tile_pool(name="cin", bufs=1))
        gpool = ctx.enter_context(tc.tile_pool(name="gates", bufs=3))
        opool = ctx.enter_context(tc.tile_pool(name="outs", bufs=3))
        pspool = ctx.enter_context(tc.tile_pool(name="ps", bufs=1, space="PSUM"))

        ps = [
            pspool.tile([P, JW], F32, tag=f"p{i}", name=f"p{i}") for i in range(8)
        ]

        # --- warmup: keep the PE activity monitor busy from preamble end so
        # the clock is at 2.4GHz when real matmuls start. Depends only on a
        # tiny memset, not on any DMA.
        wu = wpool.tile([P, P], BF16, tag="wu", name="wu")
        nc.vector.memset(wu[:], 0.0)
        # Preload the Sigmoid/Tanh activation table now: the lazy
        # ACT_TABLE_LOAD costs 1.3us on the ScalarE right when the first
        # PSUM eviction is on the critical path.
        wua = wpool.tile([P, 2], F32, tag="wua", name="wua")
        nc.scalar.activation(wua[:], wu[:, :2], AF.Sigmoid)
        # ~56 cold N=128 MMs ~= 6us busy: bridges from preamble end until
        # the first data chunks land (~13us) with the HAM already at 8/8.
        for _ in range(56):
            nc.tensor.matmul(ps[7][:, :P], wu[:], wu[:], start=True, stop=True)

        # --- SBUF tiles. xh chunk 0 is split (k0-1 / k2-7) and c chunks are
        # split in r-halves so early consumers don't wait on whole tiles.
        xh0a = xpool.tile([P, 2 * JW], BF16, tag="xh0a", name="xh0a")
        xh0b = xpool.tile([P, 2 * JW], BF16, tag="xh0b", name="xh0b")
        xh0c = xpool.tile([P, 4 * JW], BF16, tag="xh0c", name="xh0c")
        xh_t = [None] + [
            xpool.tile([P, NK * JW], BF16, tag=f"xh{nb}", name=f"xh{nb}")
            for nb in range(1, nbn)
        ]
        c_t = [
            [
                cpool.tile([P, 2 * JW], F32, tag=f"c{nb}_{rh}", name=f"c{nb}_{rh}")
                for rh in range(2)
            ]
            for nb in range(nbn)
        ]
        bias_t = wpool.tile([P, 32 * NM], F32, tag="bias", name="bias")

        def xh_sl(nb, k):
            if nb == 0:
                t, o = (xh0a, 0) if k < 2 else (xh0b, 2) if k < 4 else (xh0c, 4)
                return t[:, (k - o) * JW : (k - o + 1) * JW]
            return xh_t[nb][:, k * JW : (k + 1) * JW]

        # --- DMA issue plan (queue order = priority; lines are 2-8KB).
        # Critical window carries ONLY phase A's diet: xh0+bias+c0a on the
        # scalar queue, weight k-chunks alternating sync/gpsimd (a double
        # bandwidth share). Everything else queues behind the weights
        # (in-order drain = free prioritization).
        nc.scalar.dma_start(xh0a[:], xh_in[0][:, : 2 * JW])
        nc.scalar.dma_start(xh0b[:], xh_in[0][:, 2 * JW : 4 * JW])
        nc.scalar.dma_start(xh0c[:], xh_in[0][:, 4 * JW :])
        nc.scalar.dma_start(bias_t[:], bias_in[:])
        nc.scalar.dma_start(c_t[0][0][:], c_in[0][:, : 2 * JW])
        wts = [[None] * 2 for _ in range(NK)]
        for rh in range(2):
            for k in range(NK):
                wt = wpool.tile(
                    [P, 8 * P], BF16, tag=f"w{k}_{rh}", name=f"w{k}_{rh}"
                )
                wts[k][rh] = wt
                eng = nc.sync if k % 2 == 0 else nc.gpsimd
                eng.dma_start(wt[:], wt_in[k, rh])
        # late inputs behind the weights, earliest-needed first
        late = [(nc.sync, c_t[0][1], c_in[0][:, 2 * JW :])]
        for nb in range(1, nbn):
            e1, e2 = (nc.gpsimd, nc.sync) if nb % 2 else (nc.sync, nc.gpsimd)
            late.append((e1, xh_t[nb], xh_in[nb]))
            late.append((e2, c_t[nb][0], c_in[nb][:, : 2 * JW]))
            late.append((e1, c_t[nb][1], c_in[nb][:, 2 * JW :]))
        for eng, dst, src in late:
            eng.dma_start(dst[:], src)

        def mm_burst(nb, r, gg, base):
            """8 accumulating matmuls (k-inner) for gate gg of (nb, r)."""
            rh, rl = r // 2, r % 2
            col = (gg * 2 + rl) * P
            for k in range(NK):
                nc.tensor.matmul(
                    ps[base + gg][:],
                    wts[k][rh][:, col : col + P],
                    xh_sl(nb, k),
                    start=(k == 0),
                    stop=(k == NK - 1),
                )

        def mm_group_kouter(nb, r, base):
            """k-outer group: consumes weight chunks in arrival order."""
            rh, rl = r // 2, r % 2
            for k in range(NK):
                for gg in GATE_ORDER:
                    col = (gg * 2 + rl) * P
                    nc.tensor.matmul(
                        ps[base + gg][:],
                        wts[k][rh][:, col : col + P],
                        xh_sl(nb, k),
                        start=(k == 0),
                        stop=(k == NK - 1),
                    )

        def mm_rpair_kouter(nb, rh):
            """k-outer phase over r-pair (2rh, 2rh+1) across all 8 PSUM
            banks: consumes weight chunks in arrival order. Gate g of
            r-local rl -> bank rl*4+g."""
            for k in range(NK):
                for gg in GATE_ORDER:
                    for rl in range(2):
                        col = (gg * 2 + rl) * P
                        nc.tensor.matmul(
                            ps[rl * 4 + gg][:],
                            wts[k][rh][:, col : col + P],
                            xh_sl(nb, k),
                            start=(k == 0),
                            stop=(k == NK - 1),
                        )

        def act_gate(nb, r, gg, base, sl, dst):
            fn = AF.Tanh if gg == 1 else AF.Sigmoid
            nc.scalar.activation(
                dst[:, sl], ps[base + gg][:, sl], fn,
                bias=bias_t[:, gg * NR + r : gg * NR + r + 1],
            )

        def elementwise(nb, r, base, split=1):
            """Cell update for group (nb, r); gates in ps[base..base+3].
            ACT eviction order f,i,g,o matches the bank-demand order of the
            next group using these banks (its bursts are f,i,g,o too)."""
            cw = JW // split
            for s in range(split):
                sl = slice(s * cw, (s + 1) * cw)
                csl = slice((r % 2) * JW + s * cw, (r % 2) * JW + (s + 1) * cw)
                cti = c_t[nb][r // 2]
                ft = gpool.tile([P, JW], F32, tag="f")
                it = gpool.tile([P, JW], F32, tag="i")
                gt = gpool.tile([P, JW], F32, tag="g")
                ot = gpool.tile([P, JW], F32, tag="o")
                t1 = gpool.tile([P, JW], F32, tag="t1")
                t2 = gpool.tile([P, JW], F32, tag="t2")
                tch = gpool.tile([P, JW], F32, tag="tch")
                cn = opool.tile([P, JW], F32, tag="cn")
                hn = opool.tile([P, JW], F32, tag="hn")
                osl = slice(r * JW + s * cw, r * JW + (s + 1) * cw)
                act_gate(nb, r, 2, base, sl, ft)
                nc.vector.tensor_mul(t2[:, sl], ft[:, sl], cti[:, csl])
                act_gate(nb, r, 0, base, sl, it)
                act_gate(nb, r, 1, base, sl, gt)
                nc.vector.tensor_mul(t1[:, sl], it[:, sl], gt[:, sl])
                nc.vector.tensor_add(cn[:, sl], t1[:, sl], t2[:, sl])
                nc.scalar.activation(tch[:, sl], cn[:, sl], AF.Tanh)
                nc.sync.dma_start(c_out[nb][:, osl], cn[:, sl])
                act_gate(nb, r, 3, base, sl, ot)
                nc.vector.tensor_mul(hn[:, sl], ot[:, sl], tch[:, sl])
                nc.sync.dma_start(h_out[nb][:, osl], hn[:, sl])

        # --- phase A: batch-chunk 0 entirely k-outer, consuming weight
        # chunks in arrival order. First the r0+r1 pair across all 8 banks,
        # then r2 alone (r0's banks, freed gate-by-gate in matching order)
        # and r3 alone (r1's banks, freed long before).
        mm_rpair_kouter(0, 0)
        elementwise(0, 0, base=0)
        mm_group_kouter(0, 2, base=0)
        elementwise(0, 1, base=4)
        elementwise(0, 2, base=0)
        mm_group_kouter(0, 3, base=4)
        elementwise(0, 3, base=4)

        # --- steady state: remaining groups, g-outer k-inner, alternating
        # PSUM halves for double buffering.
        groups = [
            (nb, r) for nb in range(1, nbn) for r in range(NR)
        ]
        for j, (nb, r) in enumerate(groups):
            base = 4 * (j % 2)
            for gg in GATE_ORDER:
                mm_burst(nb, r, gg, base)
            last = j == len(groups) - 1
            elementwise(nb, r, base, split=2 if last else 1)

    nc.compile()
    return nc


def prep_shared(Wxi, Wxg, Wxf, Wxo, Whi, Whg, Whf, Who, bias_sum):
    """wt_in [NK,P,4H] bf16 and bias_in [P,NM] f32 (gate order i,g,f,o)."""
    Wx = np.concatenate([Wxi, Wxg, Wxf, Wxo], axis=0)  # [4H, IN]
    Wh = np.concatenate([Whi, Whg, Whf, Who], axis=0)  # [4H, HID]
    WT = np.concatenate([Wx.T, Wh.T], axis=0)          # [K=1024, 4H]
    # WT[k*128+p, m*128+j] -> wt[k, rh, p, (g*2+rl)*128+j], m = g*NR+rh*2+rl
    W6 = WT.reshape(NK, P, 4, 2, 2, P)        # [k, p, g, rh, rl, j]
    wt_arr = np.ascontiguousarray(
        W6.transpose(0, 3, 1, 2, 4, 5)        # [k, rh, p, g, rl, j]
        .reshape(NK, 2, P, 8 * P)
        .astype(BF16_NP)
    )
    bias_arr = np.ascontiguousarray(
        np.tile(bias_sum.reshape(NM, P).T.astype(np.float32), (1, 32))
    )
    return wt_arr, bias_arr


def prep_core(x_s, h_s, c_s):
    """Per-core xh_in [nb,P,NK*JW] bf16 and c_in [nb,P,NR*JW] f32."""
    bl = x_s.shape[0]
    nbn = bl // JW
    xhT = np.concatenate([x_s, h_s], axis=1).T  # [K=1024, bl]
    # xhT[k*128+p, nb*JW+jw] -> xh[nb, p, k*JW+jw]
    xh_arr = np.ascontiguousarray(
        xhT.reshape(NK, P, nbn, JW).transpose(2, 1, 0, 3)
        .reshape(nbn, P, NK * JW)
        .astype(BF16_NP)
    )
    cT = c_s.T  # [HID, bl]
    c_arr = np.ascontiguousarray(
        cT.reshape(NR, P, nbn, JW).transpose(2, 1, 0, 3)
        .reshape(nbn, P, NR * JW)
        .astype(np.float32)
    )
    return xh_arr, c_arr


def post_core(arr):
    """[nb,P,NR*JW] -> [bl, HID]"""
    arr = np.asarray(arr)
    nbn = arr.size // (NR * P * JW)
    arr = arr.reshape(nbn, P, NR, JW)
    return arr.transpose(0, 3, 2, 1).reshape(nbn * JW, HID)


_NC_CACHE = {}


def _get_nc(bl=BL):
    if bl not in _NC_CACHE:
        _NC_CACHE[bl] = build_nc(bl)
    return _NC_CACHE[bl]


def make_in_maps(x, h, c, Wxi, bxi, Wxo, bxo, Wxf, bxf, Wxg, bxg,
                 Whi, bhi, Who, bho, Whf, bhf, Whg, bhg, ncores=NCORES):
    bias_sum = np.concatenate(
        [bxi + bhi, bxg + bhg, bxf + bhf, bxo + bho], axis=0
    ).astype(np.float32)
    wt_arr, bias_arr = prep_shared(Wxi, Wxg, Wxf, Wxo, Whi, Whg, Whf, Who, bias_sum)
    bl = x.shape[0] // ncores
    in_maps = []
    for i in range(ncores):
        s = slice(i * bl, (i + 1) * bl)
        xh_arr, c_arr = prep_core(
            np.asarray(x[s], np.float32),
            np.asarray(h[s], np.float32),
            np.asarray(c[s], np.float32),
        )
        in_maps.append(
            {"xh_in": xh_arr, "wt_in": wt_arr, "bias_in": bias_arr, "c_in": c_arr}
        )
    return in_maps


def kernel(x, h, c, Wxi, bxi, Wxo, bxo, Wxf, bxf, Wxg, bxg,
           Whi, bhi, Who, bho, Whf, bhf, Whg, bhg):
    args = dict(
        x=np.asarray(x, np.float32), h=np.asarray(h, np.float32),
        c=np.asarray(c, np.float32),
        Wxi=np.asarray(Wxi, np.float32), bxi=np.asarray(bxi, np.float32),
        Wxo=np.asarray(Wxo, np.float32), bxo=np.asarray(bxo, np.float32),
        Wxf=np.asarray(Wxf, np.float32), bxf=np.asarray(bxf, np.float32),
        Wxg=np.asarray(Wxg, np.float32), bxg=np.asarray(bxg, np.float32),
        Whi=np.asarray(Whi, np.float32), bhi=np.asarray(bhi, np.float32),
        Who=np.asarray(Who, np.float32), bho=np.asarray(bho, np.float32),
        Whf=np.asarray(Whf, np.float32), bhf=np.asarray(bhf, np.float32),
        Whg=np.asarray(Whg, np.float32), bhg=np.asarray(bhg, np.float32),
    )
    in_maps = make_in_maps(**args)
    nc = _get_nc(BL)
    res = run_bass_kernel_spmd(nc, in_maps, core_ids=list(range(NCORES)))
    h_new = np.empty((B_FULL, HID), np.float32)
    c_new = np.empty((B_FULL, HID), np.float32)
    for i in range(NCORES):
        s = slice(i * BL, (i + 1) * BL)
        h_new[s] = post_core(res.results[i]["h_out"])
        c_new[s] = post_core(res.results[i]["c_out"])
    return (h_new, c_new)
